# revision 37
# baseline (speedup 1.0000x reference)
"""Trainium2 Bass kernel for BiLinearLayerV2.

  biLinear[b,f,d] = sum_e feature[b,f,e] * weight[f,e,d]
  out[b,f,g,d]    = biLinear[b,f,d] * feature[b,g,d] * weightLeft[f,g]

Shapes: feature [512,64,32] f32, weight [64,32,32], weightLeft [64,64].
Output [512,64,64,32] f32 (256 MB).  Data-parallel over batch: 64 b per
core x 8 cores.

The key trade: the pipeline is a pure product (no cancellation), so the
output is stored as fp16 scaled by 2^16 and the host upcasts to f32 --
halves HBM stores to 16.8 MB/core (max rel err ~1.8e-3 vs the 2e-2
gate).  That moves the bottleneck from the store DMA (~320 GB/s/core)
to the DVE: every output element must pass through one
scalar_tensor_tensor (fp32-PSUM source -> 1x mode, 120+FD cycles @
0.96 GHz; the 2x packed modes never engage on this silicon, and no
other engine can both read PSUM and multiply tensors).  The kernel is
therefore built around one fully packed 64 x [128,1024]-STT vector
stream (~73 us), with everything else scheduled to stay off it:

  1. phase_a (PE, fp32): bilinT in a block layout st[32r+d', 64h+b],
     ACT-cast to fp16 (x2^5) on ScalarE; each 64-row half is one
     Q-stationary.
  2. Q[b+64h,(g,d)] = bilinT*wl via PE against the moving operand
     R[(r,d'),(g,d)] = mask(d'==d)*wL[4j4+r,g] (fp16, x2^10).
     R0 is host-prebuilt (load rides the idle early sync ring), R1 is
     built on the DVE before its STT stream starts, R2-R15 on gpsimd
     (whose IRAM lib only lands ~16 us in; it then saturates on R
     builds until ~63 us).
  3. STT: out16 = (Q * 2) * feat  (PSUM -> fp16 SBUF), 1214 ns/tile.
  4. Stores: one 256 KB DMA per (f-pair, g-half) into an f-major DRAM
     layout (rows (f,b) = ot partitions, "(f b) c" merged AP); host
     transposes back.  s=0 on sync, s=1 on scalar; the last pair is
     split across both rings to drain in parallel.

Startup (~14 us to the first STT) is early-DMA-latency-bound
(~100 GB/s): the load order and ring assignment below are tuned so the
chain preamble -> c32 q0-rows -> phase_a(0) -> stat(0) + r0 -> Q(0) ->
STT(0) has no avoidable waits.  Measured ~91 us (baseline f32-store
design: 129 us).  NOTE: the Tile scheduler is extremely sensitive --
several "obvious" improvements (splitting loads, moving builds between
engines, host-precomputing more of the startup) all regressed by
4-17 us; measure 3+ warm runs before trusting any change.

fp16 ranges: wl x2^10, bilinT x2^5, Q carries 2^15, stored out 2^16;
all values stay in fp16 normal range (|out|*2^16 < ~4.4e3).
"""

import sys

if "/opt/trn_rl_repo" not in sys.path:
    sys.path.insert(0, "/opt/trn_rl_repo")

import numpy as np

B, F, E = 512, 64, 32
NCORES = 8
BLOC = B // NCORES  # 64
GD = F * E  # 2048
SCALE_WL = 1024.0  # 2^10 on weightLeft
SCALE_BL = 32.0  # 2^5 on biLinT (fp16 cast)
SCALE_OUT = 65536.0  # stored out = out_true * 2^16 (fp16)
PP_SCALAR = SCALE_OUT / (SCALE_WL * SCALE_BL)  # 2.0

RBUILD = "gpsimd"

_cached = {}


def _build_nc(rbuild=RBUILD):
    from contextlib import ExitStack

    import concourse.bass as bass
    import concourse.tile as tile
    from concourse import bacc, mybir

    f32 = mybir.dt.float32
    f16 = mybir.dt.float16
    nc = bacc.Bacc("TRN2", target_bir_lowering=False, debug=False)

    c32 = nc.dram_tensor(
        "c32", (128, 16 * BLOC + 16 * E), f32, kind="ExternalInput"
    ).ap()
    c16 = nc.dram_tensor(
        "c16", (128, E + 16 * F + GD), f16, kind="ExternalInput"
    ).ap()
    r0 = nc.dram_tensor("r0", (128, GD), f16, kind="ExternalInput").ap()
    # f-major layout so one DMA covers an f-pair: rows (f,b) match the
    # ot tile partition order; host transposes back
    out = nc.dram_tensor("out", (F, BLOC, GD), f16, kind="ExternalOutput").ap()

    with tile.TileContext(nc) as tc, ExitStack() as ctx:
        consts = ctx.enter_context(tc.tile_pool(name="consts", bufs=1))
        c32_t = consts.tile([128, 16 * BLOC + 16 * E], f32)
        c16_t = consts.tile([128, E + 16 * F + GD], f16)
        # load order is the startup critical path. sync: c32 q0 rows
        # (phase_a j4 0-3) -> r0 (first Q matmuls). scalar: mask+wlrep
        # head (vector R1 build) -> featd halves (first STTs) -> wlrep
        # rest (gpsimd R2+ can't start before its ~16us IRAM lib load).
        fd0 = E + 16 * F
        hd = E + 3 * F
        nc.sync.dma_start(c32_t[0:32, :], c32[0:32, :])
        nc.scalar.dma_start(c16_t[:, 0:hd], c16[:, 0:hd])
        nc.scalar.dma_start(
            c16_t[:, fd0 : fd0 + 1024], c16[:, fd0 : fd0 + 1024]
        )
        nc.scalar.dma_start(
            c16_t[:, fd0 + 1024 :], c16[:, fd0 + 1024 :]
        )
        nc.scalar.dma_start(c16_t[:, hd:fd0], c16[:, hd:fd0])
        # c32 rows 32:128 ride the scalar ring so they don't packet-
        # interleave with r0 on sync (r0 gates the first Q matmuls);
        # phase_a(4) first needs these rows ~20us in
        nc.scalar.dma_start(c32_t[32:128, :], c32[32:128, :])
        featT_t = c32_t[:, 0 : 16 * BLOC]
        wstat_t = c32_t[:, 16 * BLOC : 16 * BLOC + 16 * E]
        mask_t = c16_t[:, 0:E]
        wlrep_t = c16_t[:, E : E + 16 * F]
        featd_t = c16_t[:, E + 16 * F : E + 16 * F + GD]

        reng = nc.vector if RBUILD == "vector" else nc.gpsimd
        with (
            tc.tile_pool(name="psb", bufs=2, space=bass.MemorySpace.PSUM) as psb,
            tc.tile_pool(name="rt", bufs=6) as rtp,
            tc.tile_pool(name="stat", bufs=3) as statp,
            tc.tile_pool(name="psq", bufs=3, space=bass.MemorySpace.PSUM) as psq,
            tc.tile_pool(name="ot", bufs=8) as otp,
        ):
            stats = {}

            def phase_a(j4):
                # st[32r+d', 64h+b] = bilin[b, 4j4+2s+h, d'] for r=2s+h;
                # each 64-row half is one Q-stationary (64-deep contraction)
                tps = [
                    psb.tile([128, 512], f32, name=f"tps{i}", tag="tps")
                    for i in range(2)
                ]
                for r in range(4):
                    f = 4 * j4 + r
                    q, fq = f // 16, f % 16
                    s, h = r // 2, r % 2
                    qsl = slice(32 * q, 32 * q + 32)
                    nc.tensor.matmul(
                        tps[s][32 * r : 32 * r + 32, 64 * h : 64 * h + 64],
                        wstat_t[qsl, 32 * fq : 32 * fq + 32],
                        featT_t[qsl, fq * BLOC : (fq + 1) * BLOC],
                        start=True,
                        stop=True,
                        tile_position=(32 * q, 32 * r),
                    )
                st = statp.tile([128, 128], f16, name="stat")
                for s in range(2):
                    nc.scalar.activation(
                        st[64 * s : 64 * s + 64, :],
                        tps[s][64 * s : 64 * s + 64, 0:128],
                        mybir.ActivationFunctionType.Copy,
                        scale=SCALE_BL,
                    )
                stats[j4] = st

            mask3 = mask_t.unsqueeze(1).broadcast_to((128, F, E))
            rts = {}

            def build_r(j4, eng):
                # R[(r,d'),(g,d)] = mask(d'==d) * wL[4j4+r, g] * 2^10
                rt = rtp.tile([128, GD], f16, name="rt", tag="rt")
                wl3 = (
                    wlrep_t[:, j4 * F : (j4 + 1) * F]
                    .unsqueeze(2)
                    .broadcast_to((128, F, E))
                )
                eng.tensor_tensor(
                    rt[:].rearrange("p (g d) -> p g d", d=E),
                    mask3,
                    wl3,
                    mybir.AluOpType.mult,
                )
                rts[j4] = rt

            rt_first = rtp.tile([128, GD], f16, name="rt", tag="rt")
            nc.sync.dma_start(rt_first[:], r0)
            rts[0] = rt_first
            # R1 on DVE (idle until its first STT ~14us anyway)
            build_r(1, nc.vector)
            phase_a(0)
            phase_a(1)
            for j4 in range(16):
                if j4 == 0:
                    # gpsimd's first op fires right after its ~17us IRAM
                    # lib load; R2 is needed at ~21us, R3+ at ~26us+
                    build_r(2, reng)
                if j4 + 3 < 16:
                    build_r(j4 + 3, reng)
                if j4 + 2 < 16:
                    phase_a(j4 + 2)
                rt = rts.pop(j4)
                st = stats.pop(j4)
                for s in range(2):
                    f0 = 4 * j4 + 2 * s
                    ssl = slice(64 * s, 64 * s + 64)
                    eng = nc.sync if s == 0 else nc.scalar
                    last = j4 == 15 and s == 1
                    for cc in range(2):
                        csl = slice(1024 * cc, 1024 * (cc + 1))
                        pq = psq.tile([128, 1024], f32, name="pq", tag="pq")
                        for n in range(2):
                            nsl = slice(1024 * cc + 512 * n, 1024 * cc + 512 * (n + 1))
                            for half in range(2):
                                r = 2 * s + half
                                rsl = slice(32 * r, 32 * r + 32)
                                nc.tensor.matmul(
                                    pq[64 * half : 64 * half + 64, 512 * n : 512 * (n + 1)],
                                    st[rsl, 64 * half : 64 * half + 64],
                                    rt[rsl, nsl],
                                    start=True,
                                    stop=True,
                                    tile_position=(32 * r, 64 * half),
                                )
                        ot = otp.tile([128, 1024], f16)
                        # out16 = (Q*2)*feat on DVE (1x mode, PSUM src)
                        nc.vector.scalar_tensor_tensor(
                            ot[:],
                            pq[:],
                            PP_SCALAR,
                            featd_t[:, csl],
                            op0=mybir.AluOpType.mult,
                            op1=mybir.AluOpType.mult,
                        )
                        seng = (nc.sync if cc == 0 else nc.scalar) if last else eng
                        seng.dma_start(
                            out[f0 : f0 + 2, :, csl].rearrange(
                                "f b c -> (f b) c"
                            ),
                            ot[:],
                        )

    nc.compile()
    return nc


def _get_nc(rbuild=RBUILD):
    if rbuild not in _cached:
        _cached[rbuild] = _build_nc(rbuild)
    return _cached[rbuild]


def _host_inputs(feature, weight, weightLeft, rbuild=RBUILD):
    feature = np.ascontiguousarray(feature, dtype=np.float32)
    weight = np.ascontiguousarray(weight, dtype=np.float32)
    weightLeft = np.ascontiguousarray(weightLeft, dtype=np.float32)

    wstat = np.ascontiguousarray(weight.transpose(1, 0, 2).reshape(E, F * E))
    wl4 = (weightLeft * np.float32(SCALE_WL)).reshape(16, 4, F)
    wlrep = np.broadcast_to(
        wl4.transpose(1, 0, 2)[:, None, :, :], (4, E, 16, F)
    ).reshape(128, 16 * F)
    wlrep = np.ascontiguousarray(wlrep.astype(np.float16))
    maskc = np.ascontiguousarray(np.tile(np.eye(E, dtype=np.float16), (4, 1)))

    wstat4 = (
        wstat.reshape(E, 4, 16 * E).transpose(1, 0, 2).reshape(128, 16 * E)
    )
    r0h = np.ascontiguousarray(
        (
            wlrep[:, 0:F].astype(np.float32)[:, :, None]
            * maskc.astype(np.float32)[:, None, :]
        )
        .reshape(128, GD)
        .astype(np.float16)
    )
    in_maps = []
    for c in range(NCORES):
        fc = feature[c * BLOC : (c + 1) * BLOC]
        featd = np.tile(fc.reshape(BLOC, GD), (2, 1)).astype(np.float16)
        featT = fc.transpose(2, 1, 0).reshape(E, F * BLOC)
        featT4 = (
            featT.reshape(E, 4, 16 * BLOC)
            .transpose(1, 0, 2)
            .reshape(128, 16 * BLOC)
        )
        c32 = np.ascontiguousarray(np.concatenate([featT4, wstat4], axis=1))
        c16 = np.ascontiguousarray(
            np.concatenate([maskc, wlrep, featd], axis=1)
        )
        in_maps.append({"c32": c32, "c16": c16, "r0": r0h})
    return in_maps


def _postprocess(res):
    out16 = np.concatenate(
        [
            r["out"].reshape(F, BLOC, F, E).transpose(1, 0, 2, 3)
            for r in res.results
        ],
        axis=0,
    )
    return out16.astype(np.float32) * np.float32(1.0 / SCALE_OUT)


def _run(in_maps, trace=False, tmpdir=None, rbuild=RBUILD):
    from concourse.bass_utils import run_bass_kernel_spmd

    nc = _get_nc(rbuild)
    return run_bass_kernel_spmd(
        nc, in_maps, core_ids=list(range(NCORES)), trace=trace, tmpdir=tmpdir
    )


def kernel(feature, weight, weightLeft):
    in_maps = _host_inputs(feature, weight, weightLeft)
    res = _run(in_maps)
    return _postprocess(res)
